# revision 33
# baseline (speedup 1.0000x reference)
"""Trainium2 Bass kernel: causal attention with weight-normed QKV projections.

Problem (hardcoded): B=8, Cq=Ck=256, C=512, H=W=32 -> S=1024, N_HEAD=8, dh=64.
Sharding: pure data-parallel over batch (8 batches -> 8 cores), weights
replicated. No collectives.

Host-side glue pre-computes the weight-norm projection matrices
w^T = (g/||v_row|| * v).T in bf16 (weight preprocessing, O(C*Cin) work) and
pre-arranges every payload partition-contiguously so each input DMA is a
full-rate bulk copy spread over the three DMA-capable rings (sync/gpsimd/
scalar): wt = [wqT;wkT;wvT], qh/kh = q/k halves (bf16), gb = [bq;bk],
bv pre-broadcast [128,C] bf16, msk = [triu-ones-incl-diag | -1e9*I].

Per-core pipeline (batch b):
  0. Startup: the ACT exp table-set loads and ~20 dummy wide matmuls run
     while the input DMAs land, so the PE's HAM clock gate reaches K=8/8
     (2.4 GHz) before the first projection matmul.
  1. Fully interleaved main phase, one C-tile (= one head pair) at a time.
     Projections for pair p+1 and the V projection are emitted as FILLER
     work inside pair p's attention loop: the PE executes its queue
     in-order, so dense independent matmuls between QK bursts both hide
     the exp-wait stalls and keep the HAM clock warm. Each loop iteration
     emits the previous pair's PV group BEFORE the next QK burst for the
     same reason.
  2. Attention per head pair: K=64 QK matmuls row-group packed (the two
     heads live at partitions 0-63 / 64-127 of their C-tile; the hardware
     runs both heads' matmuls concurrently in disjoint row groups),
     per-head [128, w] logit psum tiles (lt0/lt1 ping-pong), exp per head
     straight out of PSUM into a shared bf16 e-tile. Strictly causal:
     only the lower triangle of [128,128] tiles is computed; each diagonal
     tile's mask is seeded in-psum by a uincl^T @ (-1e9*I) matmul that the
     QK chunk then accumulates onto (start=False accumulates where
     written, overwrites the rest), so exp emits pre-masked e-tiles with
     no separate mask multiply anywhere. No max subtraction: exp args are
     O(7), fine in fp32->bf16. (fp8 e-tiles were tried and are
     numerically infeasible: e4m3's 13-ln-unit dynamic range is narrower
     than this data's 13.05-unit logit spread, e5m2's 2-bit mantissa
     gives 3.7e-2 output error.)
  3. PV: out[SqTile, 65] accumulated over S_k tiles with e^T slices as
     the stationary operand (no transposes anywhere), the ones column of
     VP yielding softmax denominators for free; both heads' rows
     normalized by ONE DVE tensor_tensor against the 0-stride-broadcast
     reciprocal of column 64.
  4. Output stored [S, C] bf16 (tolerance 2e-2 >> bf16 rounding), DMA'd
     over all three rings; host casts up and transposes to [C, H, W].

Measured on trn2: 86.2-86.8us (from a 113us baseline), rel err 6.1e-3.
Engine busy: PE ~60us (in-order queue is the pacer), ACT ~45us (48 exp
calls ~= 31us streaming floor at 1 elem/lane/cycle + ~0.2us/call), DVE
~31us, GPSIMD ~9us. Known residual: the PE HAM clock re-throttles to
1.2 GHz mid-kernel during LDWEIGHTS-heavy PV stretches (small-N matmuls
don't register enough activity); feeding it dummy wide matmuls keeps it
warm but costs as much as it saves (~3us either way).
"""

import numpy as np

import concourse.bass as bass
import concourse.tile as tile
from concourse import bacc, mybir
from concourse.bass_utils import run_bass_kernel_spmd

F32 = mybir.dt.float32
BF16 = mybir.dt.bfloat16
AF = mybir.ActivationFunctionType

S = 1024          # sequence length (32*32)
CIN = 256         # input channels (Cq = Ck)
C = 512           # projection channels
NH = 8            # heads
DH = 64           # head dim
HW = 32           # spatial H = W
N_CORES = 8


def _build_module():
    nc = bacc.Bacc("TRN2", target_bir_lowering=False)

    # all inputs pre-arranged on host so every DMA is a contiguous
    # 128-partition bulk copy (the strided rearrange path runs at ~1/4 rate)
    wt_d = nc.dram_tensor("wt", [128, 6 * C], BF16, kind="ExternalInput").ap()
    qh_d = nc.dram_tensor("qh", [128, 2 * S], BF16, kind="ExternalInput").ap()
    kh_d = nc.dram_tensor("kh", [128, 2 * S], BF16, kind="ExternalInput").ap()
    gb_d = nc.dram_tensor("gb", [2, C], F32, kind="ExternalInput").ap()
    bv_d = nc.dram_tensor("bv", [128, C], BF16, kind="ExternalInput").ap()
    msk_d = nc.dram_tensor("msk", [128, 128], BF16, kind="ExternalInput").ap()
    o_d = nc.dram_tensor("o", [S, C], BF16, kind="ExternalOutput").ap()

    with tile.TileContext(nc) as tc:
        with (
            tc.tile_pool(name="const", bufs=1) as const,
            tc.tile_pool(name="persist", bufs=1) as persist,
            tc.tile_pool(name="smalls", bufs=4) as smalls,
        ):
            # PE warm-up source with no DMA dependency
            wsrc = const.tile([128, 512], BF16, name="wsrc")
            nc.vector.memset(wsrc, 0.5)

            # ---- merged input DMAs, spread across three rings so the
            # transfers land in parallel (~11us instead of ~15us)
            # wt blocks: [wq kc0, wq kc1, wk kc0, wk kc1, wv kc0, wv kc1]
            wt_sb = persist.tile([128, 6, C], BF16, tag="wt", name="wt_sb")
            nc.sync.dma_start(out=wt_sb,
                              in_=wt_d.rearrange("p (n c) -> p n c", n=6))
            msk_sb = const.tile([128, 128], BF16, name="msk_sb")
            nc.gpsimd.dma_start(out=msk_sb, in_=msk_d)
            triu = msk_sb               # strict-upper ones (causal mask)
            qhalves = []
            q_t = persist.tile([128, 2, S], BF16, tag="qk0", name="qk0")
            nc.gpsimd.dma_start(out=q_t,
                                in_=qh_d.rearrange("p (n i) -> p n i", n=2))
            k_t = persist.tile([128, 2, S], BF16, tag="qk1", name="qk1")
            nc.scalar.dma_start(out=k_t[:, 0, :], in_=kh_d[:, 0:S])
            nc.sync.dma_start(out=k_t[:, 1, :], in_=kh_d[:, S:2 * S])
            qhalves = [q_t, k_t]

            # ---- ACT exp table pre-load during the DMA wait (after the
            # scalar ring's DMA trigger so the transfer starts first)
            warm = smalls.tile([128, 1], F32, tag="warm", name="warm")
            nc.vector.memset(warm, 0.0)

            warme = smalls.tile([128, 1], BF16, tag="warme", name="warme")
            nc.scalar.activation(out=warme, in_=warm, func=AF.Exp)
            qT = [qhalves[0][:, 0, :], qhalves[0][:, 1, :]]
            kTt = [qhalves[1][:, 0, :], qhalves[1][:, 1, :]]
            gb_sb = const.tile([128, 8], F32, name="gb_sb")
            nc.scalar.dma_start(out=gb_sb,
                              in_=gb_d.rearrange("n (c p) -> p (n c)", p=128))
            bq_sb = gb_sb[:, 0:4]
            bk_sb = gb_sb[:, 4:8]
            bvb = const.tile([128, C], BF16, name="bvb")
            nc.sync.dma_start(out=bvb, in_=bv_d)

            QT, KT, VP = [], [], []
            OUT = [persist.tile([128, C], BF16, tag=f"OUT{i}", name=f"OUT{i}")
                   for i in range(8)]

            with (
                tc.tile_pool(name="psL", bufs=1, space="PSUM") as psL,
                tc.tile_pool(name="psW", bufs=1, space="PSUM") as psW,
            ):
                for ct in range(4):
                    QT.append(persist.tile([128, S], BF16, tag=f"QT{ct}", name=f"QT{ct}"))
                    KT.append(persist.tile([128, S], BF16, tag=f"KT{ct}", name=f"KT{ct}"))
                for st in range(8):
                    VP.append(persist.tile([128, NH * 65], BF16, tag=f"VP{st}",
                                           name=f"VP{st}"))

                # ---- PE warm-up: dummy matmuls (no DMA dependency) while
                # the wt/qk DMAs land, so the HAM clock gate is counting
                # toward K=8/8 (2.4 GHz) before the first projection matmul.
                wpp = psW.tile([128, 512], F32, tag="pp", bufs=2,
                               name="warm_pp")
                for wi in range(20):
                    nc.tensor.matmul(wpp, lhsT=wsrc[:, 0:128],
                                     rhs=wsrc, start=True, stop=True)

                def emit_proj_group(ct, g):
                    # g in 0..3 -> (q/k, n-half)
                    dst, wbase, src, b_sb, pnm = (
                        (QT, 0, qT, bq_sb, "q"),
                        (KT, 2, kTt, bk_sb, "k"),
                    )[g // 2]
                    n = g % 2
                    pp = psW.tile([128, 512], F32, tag="pp", bufs=2,
                                  name=f"pp{pnm}{ct}_{n}")
                    for kc in range(2):
                        nc.tensor.matmul(
                            pp,
                            lhsT=wt_sb[:, wbase + kc, 128 * ct:128 * (ct + 1)],
                            rhs=src[kc][:, 512 * n:512 * (n + 1)],
                            start=(kc == 0), stop=(kc == 1),
                        )
                    # fused bias epilogue (weight-norm scale folded on host)
                    nc.vector.tensor_scalar_add(
                        out=dst[ct][:, 512 * n:512 * (n + 1)],
                        in0=pp,
                        scalar1=b_sb[:, ct:ct + 1],
                    )

                def emit_proj(ct):
                    for g in range(4):
                        emit_proj_group(ct, g)

                def emit_v(st):
                    vp = VP[st]
                    ppv = psW.tile([128, 512], F32, tag="pp", bufs=2, name=f"ppv{st}")
                    for kc in range(2):
                        nc.tensor.matmul(
                            ppv,
                            lhsT=kTt[kc][:, 128 * st:128 * (st + 1)],
                            rhs=wt_sb[:, 4 + kc, :],
                            start=(kc == 0), stop=(kc == 1),
                        )
                    vp3 = vp.rearrange("p (h c) -> p h c", c=65)
                    nc.gpsimd.memset(vp3[:, :, 64:65], 1.0)
                    nc.vector.tensor_add(
                        vp3[:, :, 0:64],
                        ppv.rearrange("p (h c) -> p h c", c=64),
                        bvb.rearrange("p (h c) -> p h c", c=64),
                    )

                # one shared logits psum tile for all groups: group
                # j=0 (w=1024) writes all 4 banks on the first pair, so the
                # merged exp's read of the [w,1024) gap always sees
                # previously-written memory of this same tensor
                lt_sh = psL.tile([128, 2, 1024], F32, name="lt_shared")
                with (
                    tc.tile_pool(name="psPV", bufs=2, space="PSUM") as psPV,
                    tc.tile_pool(name="epool", bufs=2) as epool,
                ):
                    def emit_L(a2, j, eTs):
                        # j >= 4: two consecutive j's share one psum tile and
                        # one exp per head (ACT per-op overhead ~0.2us)
                        js = [j] if j < 4 else [j, j + 1]
                        njs_ = [S - 128 * jj for jj in js]
                        w = sum(njs_)
                        # both heads in ONE psum tile (head stride 1024 =
                        # 2 banks) and ONE exp over the contiguous range
                        # [0, 1024+w) -- the garbage gap [w, 1024) costs
                        # ~0.4us/pair of ACT streaming but halves the exp
                        # call count and the PE<->ACT handoff latency. The
                        # AP must stay 2D-contiguous (a strided 3D AP runs
                        # ~20% slower per element on ACT).
                        e = epool.tile([128, 1024 + w], BF16, tag=f"e_{j}",
                                       name=f"e_{a2}_{j}")
                        offs = []   # per j in js: (off_h0, off_h1)
                        o = 0
                        for nj_ in njs_:
                            offs.append((o, 1024 + o))
                            o += nj_
                        for oo in offs:
                            eTs.append((e, oo))
                        lt = lt_sh
                        for hi in range(2):
                            p0 = 64 * hi
                            base = 0
                            for jj, nj_ in zip(js, njs_):
                                for c0 in range(0, nj_, 512):
                                    cw = min(512, nj_ - c0)
                                    nc.tensor.matmul(
                                        lt[:, hi, base + c0:base + c0 + cw],
                                        lhsT=KT[a2][p0:p0 + 64,
                                                    128 * jj:128 * jj + 128],
                                        rhs=QT[a2][p0:p0 + 64,
                                                   128 * jj + c0:128 * jj + c0 + cw],
                                        start=True, stop=True,
                                    )
                                base += nj_
                        nc.scalar.activation(
                            out=e[:, 0:1024 + w],
                            in_=lt.rearrange("p h x -> p (h x)")[:, 0:1024 + w],
                            func=AF.Exp, scale=0.125)
                        # strictly-causal mask on diagonal tiles, on the
                        # near-idle GPSIMD engine
                        for hi in range(2):
                            for (o0, o1) in offs:
                                off = o0 if hi == 0 else o1
                                nc.gpsimd.tensor_mul(
                                    e[:, off:off + 128],
                                    e[:, off:off + 128], triu)

                    def emit_PV(a2, i, eTs):
                        # both heads accumulate into one 1-bank psum tile
                        po = psPV.tile([128, 130], F32, tag="po",
                                       name=f"po_{a2}_{i}")
                        for hi in range(2):
                            hh = 2 * a2 + hi
                            for jj in range(i + 1):
                                et, (o0, o1) = eTs[jj]
                                base = (o0, o1)[hi] + 128 * (i - jj)
                                nc.tensor.matmul(
                                    po[:, 65 * hi:65 * hi + 65],
                                    lhsT=et[:, base:base + 128],
                                    rhs=VP[jj][:, 65 * hh:65 * hh + 65],
                                    start=(jj == 0), stop=(jj == i),
                                )
                        r = smalls.tile([128, 2], F32, tag="r",
                                        name=f"r{a2}_{i}")
                        nc.vector.reciprocal(
                            r, po.rearrange("p (g x) -> p g x", g=2)[:, :, 64:65])
                        # one TT normalizes both heads: r broadcast over dh
                        r3 = bass.AP(tensor=r.tensor, offset=r.offset,
                                     ap=[list(r.ap[0]), list(r.ap[1]), [0, 64]])
                        nc.vector.tensor_mul(
                            OUT[i][:, 128 * a2:128 * a2 + 128].rearrange(
                                "p (h c) -> p h c", c=64),
                            po.rearrange("p (h c) -> p h c", c=65)[:, :, 0:64],
                            r3,
                        )

                    # proj/V groups for later pairs are emitted as FILLERS
                    # inside earlier pairs' attention loops: the PE executes
                    # its queue in-order, so dense independent matmul work
                    # between QK bursts both hides the exp-wait stalls and
                    # keeps the HAM clock gate at 2.4 GHz.
                    fillers_by_a2 = {
                        0: [(lambda g=g: emit_proj_group(1, g)) for g in range(4)]
                           + [(lambda st=st: emit_v(st)) for st in range(8)],
                        1: [(lambda g=g: emit_proj_group(2, g)) for g in range(4)],
                        2: [(lambda g=g: emit_proj_group(3, g)) for g in range(4)],
                        3: [],
                    }
                    prev_eTs = None
                    emit_proj(0)
                    for a2 in range(4):
                        fillers = list(fillers_by_a2[a2])
                        eTs = []
                        for j in range(8):
                            # PV of the previous pair FIRST: its matmuls are
                            # ready work that fills the PE queue ahead of the
                            # (possibly lt-blocked) next QK burst
                            if prev_eTs is not None:
                                emit_PV(a2 - 1, 7 - j, prev_eTs)
                            if j not in (5, 7):
                                emit_L(a2, j, eTs)
                            take = -(-len(fillers) // (8 - j))
                            for _ in range(take):
                                fillers.pop(0)()
                            if a2 == 3:
                                emit_PV(3, j, eTs)
                        prev_eTs = eTs
                    # query row 0 attends to nothing: reference zeroes it
                    nc.vector.memset(OUT[0][0:1, :], 0.0)
            rings = [nc.sync, nc.gpsimd, nc.scalar]
            for i in range(8):
                rings[i % 3].dma_start(out=o_d[128 * i:128 * (i + 1), :],
                                       in_=OUT[i])
    nc.compile()
    return nc


_CACHE = {}


def _get_module():
    if "nc" not in _CACHE:
        _CACHE["nc"] = _build_module()
    return _CACHE["nc"]


def _in_maps(inputs):
    import ml_dtypes

    q = np.asarray(inputs["query"], dtype=np.float32)
    k = np.asarray(inputs["key"], dtype=np.float32)
    B = q.shape[0]
    assert B == N_CORES
    # weight preprocessing: fold the weight-norm scale, pre-transpose
    wts = []
    for nm in ("q", "k", "v"):
        v = np.asarray(inputs[f"v{nm}"], np.float32)
        g = np.asarray(inputs[f"g{nm}"], np.float32)
        w = (g / np.linalg.norm(v, axis=1))[:, None] * v      # [C, CIN]
        wts.append(np.ascontiguousarray(w.T))                  # [CIN, C]
    # [768, C] -> [128, 6*C]: partition-contiguous for full-rate DMA
    wt = np.concatenate(wts, axis=0).reshape(6, 128, C).transpose(1, 0, 2)
    wt = np.ascontiguousarray(wt.reshape(128, 6 * C)).astype(ml_dtypes.bfloat16)
    gb = np.ascontiguousarray(np.stack(
        [np.asarray(inputs["bq"], np.float32),
         np.asarray(inputs["bk"], np.float32)]))
    bv = np.ascontiguousarray(np.broadcast_to(
        np.asarray(inputs["bv"], np.float32), (128, C))).astype(ml_dtypes.bfloat16)
    msk = np.ascontiguousarray(
        np.triu(np.ones((128, 128), np.float32), k=1).astype(ml_dtypes.bfloat16))
    shared = {"wt": wt, "gb": gb, "bv": bv, "msk": msk}

    def _half(x):   # [CIN, S] -> [128, 2*S] partition-contiguous
        return np.ascontiguousarray(
            x.reshape(2, 128, S).transpose(1, 0, 2).reshape(128, 2 * S)
        ).astype(ml_dtypes.bfloat16)

    maps = []
    for b in range(B):
        m = dict(shared)
        m["qh"] = _half(q[b].reshape(CIN, S))
        m["kh"] = _half(k[b].reshape(CIN, S))
        maps.append(m)
    return maps


def _gather(results):
    outs = []
    for b in range(N_CORES):
        o = np.asarray(results[b]["o"], dtype=np.float32)   # [S, C]
        outs.append(np.ascontiguousarray(o.T).reshape(C, HW, HW))
    return np.stack(outs).astype(np.float32)      # [B, C, H, W]


def run(inputs, **kw):
    """Run on hardware; returns (full_output, BassKernelResults)."""
    nc = _get_module()
    res = run_bass_kernel_spmd(nc, _in_maps(inputs), list(range(N_CORES)), **kw)
    return _gather(res.results), res


def kernel(**inputs):
    out, _ = run(inputs)
    return out


# revision 34
# speedup vs baseline: 1.2892x; 1.2892x over previous
"""Trainium2 Bass kernel: causal attention with weight-normed QKV projections.

Problem (hardcoded): B=8, Cq=Ck=256, C=512, H=W=32 -> S=1024, N_HEAD=8, dh=64.
Sharding: pure data-parallel over batch (8 batches -> 8 cores), weights
replicated. No collectives.

Host-side glue pre-computes the weight-norm projection matrices
w^T = (g/||v_row|| * v).T in bf16 (weight preprocessing, O(C*Cin) work) and
pre-arranges every payload partition-contiguously so each input DMA is a
full-rate bulk copy spread over the three DMA-capable rings (sync/gpsimd/
scalar): wt = [wqT;wkT;wvT], qh/kh = q/k halves (bf16), gb = [bq;bk],
bv pre-broadcast [128,C] bf16, msk = [triu-ones-incl-diag | -1e9*I].

Per-core pipeline (batch b):
  0. Startup: the ACT exp table-set loads and ~20 dummy wide matmuls run
     while the input DMAs land, so the PE's HAM clock gate reaches K=8/8
     (2.4 GHz) before the first projection matmul.
  1. Fully interleaved main phase, one C-tile (= one head pair) at a time.
     Projections for pair p+1 and the V projection are emitted as FILLER
     work inside pair p's attention loop: the PE executes its queue
     in-order, so dense independent matmuls between QK bursts both hide
     the exp-wait stalls and keep the HAM clock warm. Each loop iteration
     emits the previous pair's PV group BEFORE the next QK burst for the
     same reason.
  2. Attention per head pair: K=64 QK matmuls row-group packed (the two
     heads live at partitions 0-63 / 64-127 of their C-tile; the hardware
     runs both heads' matmuls concurrently in disjoint row groups),
     per-head [128, w] logit psum tiles (lt0/lt1 ping-pong), exp per head
     straight out of PSUM into a shared bf16 e-tile. Strictly causal:
     only the lower triangle of [128,128] tiles is computed; each diagonal
     tile's mask is seeded in-psum by a uincl^T @ (-1e9*I) matmul that the
     QK chunk then accumulates onto (start=False accumulates where
     written, overwrites the rest), so exp emits pre-masked e-tiles with
     no separate mask multiply anywhere. No max subtraction: exp args are
     O(7), fine in fp32->bf16. (fp8 e-tiles were tried and are
     numerically infeasible: e4m3's 13-ln-unit dynamic range is narrower
     than this data's 13.05-unit logit spread, e5m2's 2-bit mantissa
     gives 3.7e-2 output error.)
  3. PV: out[SqTile, 65] accumulated over S_k tiles with e^T slices as
     the stationary operand (no transposes anywhere), the ones column of
     VP yielding softmax denominators for free; both heads' rows
     normalized by ONE DVE tensor_tensor against the 0-stride-broadcast
     reciprocal of column 64.
  4. Output stored [S, C] bf16 (tolerance 2e-2 >> bf16 rounding), DMA'd
     over all three rings; host casts up and transposes to [C, H, W].

Measured on trn2: 86.2-86.8us (from a 113us baseline), rel err 6.1e-3.
Engine busy: PE ~60us (in-order queue is the pacer), ACT ~45us (48 exp
calls ~= 31us streaming floor at 1 elem/lane/cycle + ~0.2us/call), DVE
~31us, GPSIMD ~9us. Known residual: the PE HAM clock re-throttles to
1.2 GHz mid-kernel during LDWEIGHTS-heavy PV stretches (small-N matmuls
don't register enough activity); feeding it dummy wide matmuls keeps it
warm but costs as much as it saves (~3us either way).
"""

import numpy as np

import concourse.bass as bass
import concourse.tile as tile
from concourse import bacc, mybir
from concourse.bass_utils import run_bass_kernel_spmd

F32 = mybir.dt.float32
BF16 = mybir.dt.bfloat16
AF = mybir.ActivationFunctionType

S = 1024          # sequence length (32*32)
CIN = 256         # input channels (Cq = Ck)
C = 512           # projection channels
NH = 8            # heads
DH = 64           # head dim
HW = 32           # spatial H = W
N_CORES = 8


def _build_module():
    nc = bacc.Bacc("TRN2", target_bir_lowering=False)

    # all inputs pre-arranged on host so every DMA is a contiguous
    # 128-partition bulk copy (the strided rearrange path runs at ~1/4 rate)
    wt_d = nc.dram_tensor("wt", [128, 6 * C], BF16, kind="ExternalInput").ap()
    qh_d = nc.dram_tensor("qh", [128, 2 * S], BF16, kind="ExternalInput").ap()
    kh_d = nc.dram_tensor("kh", [128, 2 * S], BF16, kind="ExternalInput").ap()
    gb_d = nc.dram_tensor("gb", [2, C], F32, kind="ExternalInput").ap()
    bv_d = nc.dram_tensor("bv", [128, C], BF16, kind="ExternalInput").ap()
    msk_d = nc.dram_tensor("msk", [128, 128], BF16, kind="ExternalInput").ap()
    o_d = nc.dram_tensor("o", [S, C], BF16, kind="ExternalOutput").ap()

    with tile.TileContext(nc) as tc:
        with (
            tc.tile_pool(name="const", bufs=1) as const,
            tc.tile_pool(name="persist", bufs=1) as persist,
            tc.tile_pool(name="smalls", bufs=4) as smalls,
        ):
            # PE warm-up source with no DMA dependency
            wsrc = const.tile([128, 512], BF16, name="wsrc")
            nc.vector.memset(wsrc, 0.5)

            # ---- merged input DMAs, spread across three rings so the
            # transfers land in parallel (~11us instead of ~15us)
            # wt blocks: [wq kc0, wq kc1, wk kc0, wk kc1, wv kc0, wv kc1]
            wt_sb = persist.tile([128, 6, C], BF16, tag="wt", name="wt_sb")
            nc.sync.dma_start(out=wt_sb,
                              in_=wt_d.rearrange("p (n c) -> p n c", n=6))
            msk_sb = const.tile([128, 128], BF16, name="msk_sb")
            nc.gpsimd.dma_start(out=msk_sb, in_=msk_d)
            triu = msk_sb               # strict-upper ones (causal mask)
            qhalves = []
            q_t = persist.tile([128, 2, S], BF16, tag="qk0", name="qk0")
            nc.gpsimd.dma_start(out=q_t,
                                in_=qh_d.rearrange("p (n i) -> p n i", n=2))
            k_t = persist.tile([128, 2, S], BF16, tag="qk1", name="qk1")
            nc.scalar.dma_start(out=k_t[:, 0, :], in_=kh_d[:, 0:S])
            nc.sync.dma_start(out=k_t[:, 1, :], in_=kh_d[:, S:2 * S])
            qhalves = [q_t, k_t]

            # ---- ACT exp table pre-load during the DMA wait (after the
            # scalar ring's DMA trigger so the transfer starts first)
            warm = smalls.tile([128, 1], F32, tag="warm", name="warm")
            nc.vector.memset(warm, 0.0)

            warme = smalls.tile([128, 1], BF16, tag="warme", name="warme")
            nc.scalar.activation(out=warme, in_=warm, func=AF.Exp)
            qT = [qhalves[0][:, 0, :], qhalves[0][:, 1, :]]
            kTt = [qhalves[1][:, 0, :], qhalves[1][:, 1, :]]
            gb_sb = const.tile([128, 8], F32, name="gb_sb")
            nc.scalar.dma_start(out=gb_sb,
                              in_=gb_d.rearrange("n (c p) -> p (n c)", p=128))
            bq_sb = gb_sb[:, 0:4]
            bk_sb = gb_sb[:, 4:8]
            bvb = const.tile([128, C], BF16, name="bvb")
            nc.sync.dma_start(out=bvb, in_=bv_d)

            QT, KT, VP = [], [], []
            OUT = [persist.tile([128, C], BF16, tag=f"OUT{i}", name=f"OUT{i}")
                   for i in range(8)]

            with (
                tc.tile_pool(name="psL", bufs=1, space="PSUM") as psL,
                tc.tile_pool(name="psW", bufs=1, space="PSUM") as psW,
            ):
                for ct in range(4):
                    QT.append(persist.tile([128, S], BF16, tag=f"QT{ct}", name=f"QT{ct}"))
                    KT.append(persist.tile([128, S], BF16, tag=f"KT{ct}", name=f"KT{ct}"))
                for st in range(8):
                    VP.append(persist.tile([128, NH * 65], BF16, tag=f"VP{st}",
                                           name=f"VP{st}"))

                # ---- PE warm-up: dummy matmuls (no DMA dependency) while
                # the wt/qk DMAs land, so the HAM clock gate is counting
                # toward K=8/8 (2.4 GHz) before the first projection matmul.
                wpp = psW.tile([128, 512], F32, tag="pp", bufs=2,
                               name="warm_pp")
                for wi in range(20):
                    nc.tensor.matmul(wpp, lhsT=wsrc[:, 0:128],
                                     rhs=wsrc, start=True, stop=True)

                def emit_proj_group(ct, g):
                    # g in 0..3 -> (q/k, n-half)
                    dst, wbase, src, b_sb, pnm = (
                        (QT, 0, qT, bq_sb, "q"),
                        (KT, 2, kTt, bk_sb, "k"),
                    )[g // 2]
                    n = g % 2
                    pp = psW.tile([128, 512], F32, tag="pp", bufs=2,
                                  name=f"pp{pnm}{ct}_{n}")
                    for kc in range(2):
                        nc.tensor.matmul(
                            pp,
                            lhsT=wt_sb[:, wbase + kc, 128 * ct:128 * (ct + 1)],
                            rhs=src[kc][:, 512 * n:512 * (n + 1)],
                            start=(kc == 0), stop=(kc == 1),
                        )
                    # fused bias epilogue (weight-norm scale folded on host)
                    nc.vector.tensor_scalar_add(
                        out=dst[ct][:, 512 * n:512 * (n + 1)],
                        in0=pp,
                        scalar1=b_sb[:, ct:ct + 1],
                    )

                def emit_proj(ct):
                    for g in range(4):
                        emit_proj_group(ct, g)

                def emit_v(st):
                    vp = VP[st]
                    ppv = psW.tile([128, 512], F32, tag="pp", bufs=2, name=f"ppv{st}")
                    for kc in range(2):
                        nc.tensor.matmul(
                            ppv,
                            lhsT=kTt[kc][:, 128 * st:128 * (st + 1)],
                            rhs=wt_sb[:, 4 + kc, :],
                            start=(kc == 0), stop=(kc == 1),
                        )
                    vp3 = vp.rearrange("p (h c) -> p h c", c=65)
                    nc.gpsimd.memset(vp3[:, :, 64:65], 1.0)
                    nc.vector.tensor_add(
                        vp3[:, :, 0:64],
                        ppv.rearrange("p (h c) -> p h c", c=64),
                        bvb.rearrange("p (h c) -> p h c", c=64),
                    )

                with (
                    tc.tile_pool(name="psPV", bufs=2, space="PSUM") as psPV,
                    tc.tile_pool(name="epool", bufs=2) as epool,
                ):
                    def emit_L(a2, j, eTs):
                        # j >= 4: two consecutive j's share one psum tile and
                        # one exp per head (ACT per-op overhead ~0.2us)
                        js = [j] if j < 4 else [j, j + 1]
                        njs_ = [S - 128 * jj for jj in js]
                        w = sum(njs_)
                        e = epool.tile([128, 2 * w], BF16, tag=f"e_{j}",
                                       name=f"e_{a2}_{j}")
                        offs = []   # per j in js: (off_h0, off_h1)
                        o = 0
                        for nj_ in njs_:
                            offs.append((o, w + o))
                            o += nj_
                        for oo in offs:
                            eTs.append((e, oo))
                        for hi in range(2):
                            p0 = 64 * hi
                            lt = psL.tile([128, w], F32, tag=f"lt{hi}",
                                          name=f"lt{hi}_{a2}_{j}")
                            base = 0
                            for jj, nj_ in zip(js, njs_):
                                for c0 in range(0, nj_, 512):
                                    cw = min(512, nj_ - c0)
                                    nc.tensor.matmul(
                                        lt[:, base + c0:base + c0 + cw],
                                        lhsT=KT[a2][p0:p0 + 64,
                                                    128 * jj:128 * jj + 128],
                                        rhs=QT[a2][p0:p0 + 64,
                                                   128 * jj + c0:128 * jj + c0 + cw],
                                        start=True, stop=True,
                                    )
                                base += nj_
                            nc.scalar.activation(
                                out=e[:, hi * w:hi * w + w], in_=lt,
                                func=AF.Exp, scale=0.125)
                            # strictly-causal mask on diagonal tiles, on the
                            # near-idle GPSIMD engine (keeps the pacing PE
                            # queue free of mask matmuls)
                            for (o0, o1) in offs:
                                off = o0 if hi == 0 else o1
                                nc.gpsimd.tensor_mul(
                                    e[:, off:off + 128],
                                    e[:, off:off + 128], triu)

                    def emit_PV(a2, i, eTs):
                        # both heads accumulate into one 1-bank psum tile
                        po = psPV.tile([128, 130], F32, tag="po",
                                       name=f"po_{a2}_{i}")
                        for hi in range(2):
                            hh = 2 * a2 + hi
                            for jj in range(i + 1):
                                et, (o0, o1) = eTs[jj]
                                base = (o0, o1)[hi] + 128 * (i - jj)
                                nc.tensor.matmul(
                                    po[:, 65 * hi:65 * hi + 65],
                                    lhsT=et[:, base:base + 128],
                                    rhs=VP[jj][:, 65 * hh:65 * hh + 65],
                                    start=(jj == 0), stop=(jj == i),
                                )
                        r = smalls.tile([128, 2], F32, tag="r",
                                        name=f"r{a2}_{i}")
                        nc.vector.reciprocal(
                            r, po.rearrange("p (g x) -> p g x", g=2)[:, :, 64:65])
                        # one TT normalizes both heads: r broadcast over dh
                        r3 = bass.AP(tensor=r.tensor, offset=r.offset,
                                     ap=[list(r.ap[0]), list(r.ap[1]), [0, 64]])
                        nc.vector.tensor_mul(
                            OUT[i][:, 128 * a2:128 * a2 + 128].rearrange(
                                "p (h c) -> p h c", c=64),
                            po.rearrange("p (h c) -> p h c", c=65)[:, :, 0:64],
                            r3,
                        )

                    # proj/V groups for later pairs are emitted as FILLERS
                    # inside earlier pairs' attention loops: the PE executes
                    # its queue in-order, so dense independent matmul work
                    # between QK bursts both hides the exp-wait stalls and
                    # keeps the HAM clock gate at 2.4 GHz.
                    fillers_by_a2 = {
                        0: [(lambda g=g: emit_proj_group(1, g)) for g in range(4)]
                           + [(lambda st=st: emit_v(st)) for st in range(8)],
                        1: [(lambda g=g: emit_proj_group(2, g)) for g in range(4)],
                        2: [(lambda g=g: emit_proj_group(3, g)) for g in range(4)],
                        3: [],
                    }
                    prev_eTs = None
                    emit_proj(0)
                    for a2 in range(4):
                        fillers = list(fillers_by_a2[a2])
                        eTs = []
                        for j in range(8):
                            # PV of the previous pair FIRST: its matmuls are
                            # ready work that fills the PE queue ahead of the
                            # (possibly lt-blocked) next QK burst
                            if prev_eTs is not None:
                                emit_PV(a2 - 1, 7 - j, prev_eTs)
                            if j not in (5, 7):
                                emit_L(a2, j, eTs)
                            take = -(-len(fillers) // (8 - j))
                            for _ in range(take):
                                fillers.pop(0)()
                            if a2 == 3:
                                emit_PV(3, j, eTs)
                        prev_eTs = eTs
                    # query row 0 attends to nothing: reference zeroes it
                    nc.vector.memset(OUT[0][0:1, :], 0.0)
            rings = [nc.sync, nc.gpsimd, nc.scalar]
            for i in range(8):
                rings[i % 3].dma_start(out=o_d[128 * i:128 * (i + 1), :],
                                       in_=OUT[i])
    nc.compile()
    return nc


_CACHE = {}


def _get_module():
    if "nc" not in _CACHE:
        _CACHE["nc"] = _build_module()
    return _CACHE["nc"]


def _in_maps(inputs):
    import ml_dtypes

    q = np.asarray(inputs["query"], dtype=np.float32)
    k = np.asarray(inputs["key"], dtype=np.float32)
    B = q.shape[0]
    assert B == N_CORES
    # weight preprocessing: fold the weight-norm scale, pre-transpose
    wts = []
    for nm in ("q", "k", "v"):
        v = np.asarray(inputs[f"v{nm}"], np.float32)
        g = np.asarray(inputs[f"g{nm}"], np.float32)
        w = (g / np.linalg.norm(v, axis=1))[:, None] * v      # [C, CIN]
        wts.append(np.ascontiguousarray(w.T))                  # [CIN, C]
    # [768, C] -> [128, 6*C]: partition-contiguous for full-rate DMA
    wt = np.concatenate(wts, axis=0).reshape(6, 128, C).transpose(1, 0, 2)
    wt = np.ascontiguousarray(wt.reshape(128, 6 * C)).astype(ml_dtypes.bfloat16)
    gb = np.ascontiguousarray(np.stack(
        [np.asarray(inputs["bq"], np.float32),
         np.asarray(inputs["bk"], np.float32)]))
    bv = np.ascontiguousarray(np.broadcast_to(
        np.asarray(inputs["bv"], np.float32), (128, C))).astype(ml_dtypes.bfloat16)
    msk = np.ascontiguousarray(
        np.triu(np.ones((128, 128), np.float32), k=1).astype(ml_dtypes.bfloat16))
    shared = {"wt": wt, "gb": gb, "bv": bv, "msk": msk}

    def _half(x):   # [CIN, S] -> [128, 2*S] partition-contiguous
        return np.ascontiguousarray(
            x.reshape(2, 128, S).transpose(1, 0, 2).reshape(128, 2 * S)
        ).astype(ml_dtypes.bfloat16)

    maps = []
    for b in range(B):
        m = dict(shared)
        m["qh"] = _half(q[b].reshape(CIN, S))
        m["kh"] = _half(k[b].reshape(CIN, S))
        maps.append(m)
    return maps


def _gather(results):
    outs = []
    for b in range(N_CORES):
        o = np.asarray(results[b]["o"], dtype=np.float32)   # [S, C]
        outs.append(np.ascontiguousarray(o.T).reshape(C, HW, HW))
    return np.stack(outs).astype(np.float32)      # [B, C, H, W]


def run(inputs, **kw):
    """Run on hardware; returns (full_output, BassKernelResults)."""
    nc = _get_module()
    res = run_bass_kernel_spmd(nc, _in_maps(inputs), list(range(N_CORES)), **kw)
    return _gather(res.results), res


def kernel(**inputs):
    out, _ = run(inputs)
    return out
